# revision 1
# baseline (speedup 1.0000x reference)
"""Trainium2 Bass kernel for 2D block-local multi-head attention.

Problem (hardcoded): x [1,128,48,64] -> 3x3 conv projections to q/k/v
(d_model=32, 8 heads, d_head=4), t2t local_attention_2d with
query_shape=(128,24), memory_flange=(8,8), combine heads, 3x3 output conv.

Key structural facts exploited:
  * H=128, W=48, query blocks 128x24 -> exactly 2 blocks (nH=1, nW=2).
  * The memory flange (8 px each side) is entirely zero padding, which the
    reference masks with -1e9 (exp -> exactly 0 in fp32). So each block's
    effective key set is the static 128x32 strip of ORIGINAL pixels:
    block b queries = cols [24b, 24b+24), keys = cols [16b, 16b+32).
  * Softmax without max-subtraction is safe: logits are O(10), and bf16
    shares the fp32 exponent range, so exp cannot overflow.

Sharding: one head per NeuronCore (8 heads / 8 cores), zero cross-core
communication. Each core computes q/k/v for its head (full image), block-local
attention, and a partial output conv (contraction over its 4 head channels).
Host sums the 8 partial [64, 6144] results.

Conv trick: with channel-major tensors stored on the padded 130x50 grid, a
3x3 tap (dh, dw) is a pure flat-index shift of dh*50+dw, so the matmul RHS is
one contiguous run over padded output positions (matmul RHS must be 1-D);
the junk columns (c=48, 49) are dropped in the psum->SBUF copy. The output
conv additionally packs tap pairs (dh,0)+(dh,1) into one 8-partition
contraction using a copy of o^T pre-shifted by one column on partitions 4:8.

Attention layouts (channel-on-partition):
  logitsT psum [128 keys, G queries] = K_kt.T @ Q  (keys on partitions)
  exp tile (bf16) -> AV matmul:  av[8, q] += V'_kt.T @ exp_kt, where V'_kt
  [128 keys, 8] holds v in cols 0:4 and 1.0 in cols 4:8 (denominator rows).
Matmuls use float32r (full-rate fp32) for convs and bf16 for attention.
"""

import numpy as np

H, W, CIN, DM, NH, DH = 128, 48, 64, 32, 8, 4
HP, WP = H + 2, W + 2          # padded spatial dims for 3x3 SAME conv
PADN = HP * WP + 4             # padded flat buffer size (+4 tail overrun slack)
NPIX = H * W                   # 6144
QW, KW = 24, 32                # per-block query/key column widths
NQ = H * QW                    # 3072 queries per block
NK = H * KW                    # 4096 keys per block
NKT = 32                       # key tiles (128 keys each) per block
G = 1024                       # query granule (psum/ACT tile width)
NG = NQ // G                   # 3
CHUNK_ROWS = 8                 # conv output rows per matmul chunk
NCHUNK = H // CHUNK_ROWS       # 16
CN = CHUNK_ROWS * WP           # conv matmul free size (padded-width run), 400

_cached = {}


def _build_nc():
    import concourse.bacc as bacc
    import concourse.tile as tile
    import concourse.mybir as mybir

    f32 = mybir.dt.float32
    f32r = mybir.dt.float32r    # fp32 data, full-rate PE (reduced mul precision)
    bf16 = mybir.dt.bfloat16

    nc = bacc.Bacc("TRN2", target_bir_lowering=False)

    xx_d = nc.dram_tensor("xx", [128, PADN], bf16, kind="ExternalInput")
    wqkv_d = nc.dram_tensor("wqkv", [128, 6 * 12], bf16, kind="ExternalInput")
    bias_d = nc.dram_tensor("bias12", [12, 1], f32, kind="ExternalInput")
    wo2_d = nc.dram_tensor("wo2", [8, 3 * 64], f32r, kind="ExternalInput")
    wo1_d = nc.dram_tensor("wo1", [DH, 3 * 64], f32r, kind="ExternalInput")
    id4_d = nc.dram_tensor("id4", [DH, DH], bf16, kind="ExternalInput")
    zp_d = nc.dram_tensor("zp", [DH, PADN], f32r, kind="ExternalInput")
    outp_d = nc.dram_tensor("outp", [CIN, NPIX], f32, kind="ExternalOutput")

    with tile.TileContext(nc) as tc:
        with tc.tile_pool(name="main", bufs=1) as mp:
            xx = mp.tile([128, PADN], bf16)
            wqkv = mp.tile([128, 6 * 12], bf16)
            bias12 = mp.tile([12, 1], f32)
            wo2 = mp.tile([8, 3 * 64], f32r)
            wo1 = mp.tile([DH, 3 * 64], f32r)
            id4 = mp.tile([DH, DH], bf16)
            xx_ap = xx_d.ap()
            for q4 in range(4):
                s4 = (PADN // 4) * q4
                e4 = PADN if q4 == 3 else (PADN // 4) * (q4 + 1)
                nc.sync.dma_start(xx[:, s4:e4], xx_ap[:, s4:e4])
            nc.sync.dma_start(wqkv[:], wqkv_d.ap())
            nc.sync.dma_start(bias12[:], bias_d.ap())
            nc.sync.dma_start(wo2[:], wo2_d.ap())
            nc.sync.dma_start(wo1[:], wo1_d.ap())
            nc.sync.dma_start(id4[:], id4_d.ap())

            qkvT = mp.tile([12, NPIX], bf16)
            kTf = mp.tile([DH, NPIX], bf16)      # k^T spatial (DMA dest)
            vT = mp.tile([DH, NPIX], bf16)       # v^T spatial (DMA dest)
            kTb = mp.tile([DH, 2 * NK], bf16)    # block-contiguous key strips
            vTb = mp.tile([DH, 2 * NK], bf16)    # block-contiguous value strips
            qb = mp.tile([DH, 2 * NQ], bf16)     # block-contiguous queries
            vp = mp.tile([128, 2 * NKT * 8], bf16)  # V' tiles, ones in cols 4:8
            av_sb = mp.tile([8, 2 * NQ], f32)    # rows 0:4 unnorm o, 4:8 den
            ebias = mp.tile([128, 1], f32)       # exp input shift (overflow guard)
            actwarm = mp.tile([128, 1], f32)     # dummy exp target (table preload)
            pewarm = mp.tile([DH, 512], bf16)    # garbage src for PE HAM warmup
            den4 = mp.tile([DH, NQ], f32)        # per-block denominator staging
            oo = mp.tile([8, PADN], f32r)        # padded o^T; rows 4:8 = col+1

            # ---- q/k/v projections: 3x3 conv, tap pairs (dh,0)+(dh,1)
            # packed on 128 partitions (xx rows 64:128 are shifted by +1 col),
            # plus (dh,2) singles; bias added in the psum->SBUF copy.
            # Repacks/V' transposes are emitted per chunk, interleaved with the
            # conv, so the in-order DVE stream overlaps them with later chunks.
            nc.vector.memset(vp[:], 1.0)
            nc.vector.memset(ebias[:], -30.0)
            # dummy exp at t~0: pulls the ~2.7us ACT exp-table load off the
            # critical path (it would otherwise fire at the first real exp,
            # right when the attention pipeline starts)
            nc.scalar.activation(
                actwarm[:], ebias[:], mybir.ActivationFunctionType.Exp,
                bias=ebias[:],
            )
            # ~3.4us of dummy matmuls during the DMA-in window: drives the PE
            # HAM activity monitor to full clock (2.4GHz) before the conv, and
            # absorbs the cold-clock warmup in otherwise-idle PE time. Reads
            # uninitialized SBUF (never written - no deps), result unused.
            with tc.tile_pool(name="wps", bufs=1, space="PSUM") as wps:
                nc.vector.memset(pewarm[:], 1.0)
                wp = wps.tile([DH, 512], f32, tag="wp")
                for _ in range(6):
                    nc.tensor.matmul(wp[:], pewarm[:, 0:DH], pewarm[:],
                                     start=True, stop=True)
            qT_v = qkvT[0:4, :].rearrange("p (h w) -> p h w", w=W)
            qb_v = qb[:].rearrange("p (b h w) -> p b h w", b=2, w=QW)
            kT_v = kTf[:].rearrange("p (h w) -> p h w", w=W)
            kb_v = kTb[:].rearrange("p (b h w) -> p b h w", b=2, w=KW)
            vT_v = vT[:].rearrange("p (h w) -> p h w", w=W)
            vb_v = vTb[:].rearrange("p (b h w) -> p b h w", b=2, w=KW)
            with (
                tc.tile_pool(name="cps", bufs=2, space="PSUM") as cps,
                tc.tile_pool(name="tps", bufs=4, space="PSUM") as tps,
            ):
                for ci in range(NCHUNK):
                    ps = cps.tile([12, CN], f32, tag="cps")
                    f0 = ci * CHUNK_ROWS * WP
                    for dh in range(3):
                        s = f0 + dh * WP
                        nc.tensor.matmul(
                            ps[:], wqkv[:, 12 * dh:12 * (dh + 1)],
                            xx[:, s:s + CN],
                            start=(dh == 0), stop=False,
                        )
                        nc.tensor.matmul(
                            ps[:], wqkv[0:CIN, 36 + 12 * dh:36 + 12 * (dh + 1)],
                            xx[0:CIN, s + 2:s + 2 + CN],
                            start=False, stop=(dh == 2),
                        )
                    # bias add + drop the 2 junk columns (cast to bf16)
                    psv = ps[:].rearrange("p (r c) -> p r c", c=WP)
                    nc.vector.tensor_scalar_add(
                        qkvT[:, ci * CHUNK_ROWS * W:(ci + 1) * CHUNK_ROWS * W],
                        psv[:, :, 0:W], bias12[:])
                    r0 = ci * CHUNK_ROWS
                    rs = slice(r0 * W, (r0 + CHUNK_ROWS) * W)
                    nc.sync.dma_start(kTf[:, rs], qkvT[4:8, rs])
                    nc.sync.dma_start(vT[:, rs], qkvT[8:12, rs])
                    rr = slice(r0, r0 + CHUNK_ROWS)
                    for b in range(2):
                        nc.vector.tensor_copy(
                            qb_v[:, b, rr], qT_v[:, rr, QW * b:QW * b + QW])
                        nc.vector.tensor_copy(
                            kb_v[:, b, rr], kT_v[:, rr, 16 * b:16 * b + KW])
                        nc.vector.tensor_copy(
                            vb_v[:, b, rr], vT_v[:, rr, 16 * b:16 * b + KW])
                        for kt in (2 * ci, 2 * ci + 1):
                            ps2 = tps.tile([128, DH], bf16, tag="tps")
                            nc.tensor.transpose(
                                ps2[:],
                                vTb[:, b * NK + 128 * kt:b * NK + 128 * (kt + 1)],
                                id4[:],
                            )
                            base = (b * NKT + kt) * 8
                            nc.scalar.copy(vp[:, base:base + 4], ps2[:])

            # padded o^T borders zeroed while attention runs
            nc.sync.dma_start(oo[0:4, :], zp_d.ap())

            # ---- attention + per-block normalization ----
            oo_v = oo[0:DH, 0:HP * WP].rearrange("p (h w) -> p h w", w=WP)
            with (
                tc.tile_pool(name="lgp", bufs=3, space="PSUM") as lgp,
                tc.tile_pool(name="avp", bufs=1, space="PSUM") as avp,
                tc.tile_pool(name="exp", bufs=3) as exp_pool,
            ):
                def norm_half(b, hf):
                    # normalize rows [64*hf, 64*hf+64) of block b:
                    # o = unnorm / den, written into the padded o^T grid
                    HQ = NQ // 2
                    sl = slice(b * NQ + hf * HQ, b * NQ + (hf + 1) * HQ)
                    dn = den4[:, hf * HQ:(hf + 1) * HQ]
                    nc.sync.dma_start(dn, av_sb[4:8, sl])
                    nc.vector.reciprocal(dn, dn)
                    nc.vector.tensor_mul(av_sb[0:4, sl], av_sb[0:4, sl], dn)
                    on_v = av_sb[0:4, sl].rearrange("p (h w) -> p h w", w=QW)
                    r0 = hf * (H // 2)
                    nc.vector.tensor_copy(
                        oo_v[:, 1 + r0:1 + r0 + H // 2,
                             1 + QW * b:1 + QW * b + QW], on_v)

                for b in range(2):
                    for g in range(NG):
                        q0 = b * NQ + g * G
                        av = avp.tile([8, G], f32, tag="av")
                        for kt in range(NKT):
                            lg = lgp.tile([128, G], f32, tag="lg")
                            kap = kTb[:, b * NK + 128 * kt:b * NK + 128 * (kt + 1)]
                            for j in range(G // 512):
                                nc.tensor.matmul(
                                    lg[:, 512 * j:512 * (j + 1)],
                                    kap,
                                    qb[:, q0 + 512 * j:q0 + 512 * (j + 1)],
                                    start=True, stop=True,
                                )
                            ex = exp_pool.tile([128, G], bf16, tag="ex")
                            # bias shifts exp's overflow window to logits in
                            # (-57, +118) at zero cost (free affine stage);
                            # num/den scale identically so the result is exact
                            nc.scalar.activation(
                                ex[:], lg[:],
                                mybir.ActivationFunctionType.Exp,
                                bias=ebias[:],
                            )
                            vbase = (b * NKT + kt) * 8
                            for j in range(G // 512):
                                nc.tensor.matmul(
                                    av[:, 512 * j:512 * (j + 1)],
                                    vp[:, vbase:vbase + 8],
                                    ex[:, 512 * j:512 * (j + 1)],
                                    start=(kt == 0), stop=(kt == NKT - 1),
                                )
                        nc.vector.tensor_copy(av_sb[:, q0:q0 + G], av[:])
                        # rows [0,64) are covered by granules 0-1; rows
                        # [64,128) by granules 1-2 -> normalize early
                        if g == 1:
                            norm_half(b, 0)
                        elif g == 2:
                            norm_half(b, 1)

            # shifted copy for tap pairing: oo[4:8, c] = oo[0:4, c+1],
            # chunked by row-halves so the output conv can start early
            HF = (1 + H // 2) * WP
            nc.sync.dma_start(oo[4:8, 0:HF], oo[0:4, 1:HF + 1])
            nc.sync.dma_start(oo[4:8, HF:PADN - 1], oo[0:4, HF + 1:PADN])

            # ---- output conv (partial over this head's 4 channels) ----
            # tap pairs (dh,0)+(dh,1) via 8-partition contraction + (dh,2) singles
            outp_ap = outp_d.ap()
            with (
                tc.tile_pool(name="ops", bufs=2, space="PSUM") as ops,
                tc.tile_pool(name="ost", bufs=3) as ost,
            ):
                for ci in range(NCHUNK):
                    ps = ops.tile([CIN, CN], f32, tag="ops")
                    f0 = ci * CHUNK_ROWS * WP
                    for dh in range(3):
                        s = f0 + dh * WP
                        nc.tensor.matmul(
                            ps[:], wo2[:, 64 * dh:64 * (dh + 1)], oo[:, s:s + CN],
                            start=(dh == 0), stop=False,
                        )
                        nc.tensor.matmul(
                            ps[:], wo1[:, 64 * dh:64 * (dh + 1)],
                            oo[0:4, s + 2:s + 2 + CN],
                            start=False, stop=(dh == 2),
                        )
                    psv = ps[:].rearrange("p (r c) -> p r c", c=WP)
                    stage = ost.tile([CIN, CHUNK_ROWS * W], f32, tag="ost")
                    nc.vector.tensor_copy(stage[:], psv[:, :, 0:W])
                    nc.sync.dma_start(
                        outp_ap[:, ci * CHUNK_ROWS * W:(ci + 1) * CHUNK_ROWS * W],
                        stage[:])

    nc.compile()
    return nc


def _prep_inputs(x, wq, bq, wk, bk, wv, bv, wo):
    f32 = np.float32
    x = np.ascontiguousarray(np.asarray(x, f32))
    scale = f32(DH) ** -0.5

    bf = ml_bf16()
    xx = np.zeros((128, PADN), np.float32)
    xv = xx[:CIN, :HP * WP].reshape(CIN, HP, WP)
    xv[:, 1:1 + H, 1:1 + W] = x[0].transpose(2, 0, 1)
    xx[CIN:, :PADN - 1] = xx[:CIN, 1:]
    xx = xx.astype(bf)

    wq = np.asarray(wq, f32) * scale
    bq = np.asarray(bq, f32) * scale
    wk = np.asarray(wk, f32)
    bk = np.asarray(bk, f32)
    wv = np.asarray(wv, f32)
    bv = np.asarray(bv, f32)
    wo = np.asarray(wo, f32)

    id4 = np.eye(DH, dtype=ml_bf16())
    zp = np.zeros((DH, PADN), f32)
    in_maps = []
    for h in range(NH):
        sl = slice(4 * h, 4 * h + 4)
        wqkv = np.zeros((128, 6, 12), f32)
        for dh in range(3):
            for p, dw in ((0, 0), (1, 1)):   # pair slots on partition halves
                wqkv[64 * p:64 * p + CIN, dh, 0:4] = wq[dh, dw, :, sl]
                wqkv[64 * p:64 * p + CIN, dh, 4:8] = wk[dh, dw, :, sl]
                wqkv[64 * p:64 * p + CIN, dh, 8:12] = wv[dh, dw, :, sl]
            wqkv[:CIN, 3 + dh, 0:4] = wq[dh, 2, :, sl]
            wqkv[:CIN, 3 + dh, 4:8] = wk[dh, 2, :, sl]
            wqkv[:CIN, 3 + dh, 8:12] = wv[dh, 2, :, sl]
        bias12 = np.concatenate([bq[sl], bk[sl], bv[sl]]).reshape(12, 1)
        wo2 = np.zeros((8, 3, 64), f32)
        wo1 = np.zeros((DH, 3, 64), f32)
        for dh in range(3):
            wo2[0:4, dh] = wo[dh, 0, sl, :]
            wo2[4:8, dh] = wo[dh, 1, sl, :]
            wo1[:, dh] = wo[dh, 2, sl, :]
        in_maps.append({
            "xx": xx,
            "bias12": np.ascontiguousarray(bias12.astype(f32)),
            "wqkv": np.ascontiguousarray(wqkv.reshape(128, 6 * 12).astype(bf)),
            "wo2": np.ascontiguousarray(wo2.reshape(8, 3 * 64)),
            "wo1": np.ascontiguousarray(wo1.reshape(DH, 3 * 64)),
            "id4": id4,
            "zp": zp,
        })
    return in_maps


def ml_bf16():
    import ml_dtypes
    return ml_dtypes.bfloat16


def _run(in_maps, trace=False, trace_cores=None):
    from concourse.bass_utils import run_bass_kernel_spmd

    if "nc" not in _cached:
        _cached["nc"] = _build_nc()
    return run_bass_kernel_spmd(
        _cached["nc"], in_maps, core_ids=list(range(NH)),
        trace=trace, trace_cores=trace_cores,
    )


def kernel(x, wq, bq, wk, bk, wv, bv, wo):
    in_maps = _prep_inputs(x, wq, bq, wk, bk, wv, bv, wo)
    res = _run(in_maps)
    acc = np.zeros((CIN, NPIX), np.float64)
    for r in res.results:
        acc += r["outp"].astype(np.float64)
    out = acc.astype(np.float32).reshape(CIN, H, W).transpose(1, 2, 0)
    return out[None]



# revision 10
# speedup vs baseline: 1.3495x; 1.3495x over previous
"""Trainium2 Bass kernel for 2D block-local multi-head attention.

Problem (hardcoded): x [1,128,48,64] -> 3x3 conv projections to q/k/v
(d_model=32, 8 heads, d_head=4), t2t local_attention_2d with
query_shape=(128,24), memory_flange=(8,8), combine heads, 3x3 output conv.

Structural facts exploited:
  * H=128, W=48, query blocks 128x24 -> exactly 2 blocks (nH=1, nW=2).
  * The memory flange is entirely zero padding, masked to -1e9 by the
    reference (exp -> exactly 0), so block b's effective key set is the
    static 128x32 strip of ORIGINAL pixels: cols [16b, 16b+32).
  * bf16 exp weights keep softmax exact enough (fp8 weights and fp8 q/k
    both blow the 2e-2 budget; measured empirically).

Sharding: one head per NeuronCore, zero cross-core communication. Each
core computes q/k/v (full image), block-local attention, and a partial
output conv over its 4 head channels; host sums the 8 partial results.

Performance structure (per core):
  * Conv qkv: channel-major on the padded 130x50 grid; a 3x3 tap is a
    flat-index shift, tap pairs packed on 128 partitions (as before).
  * Logits: out = K_tile.T @ Q -> psum [128 keys, 1024 queries]; K tile
    is a strided view straight into the spatial k^T image (no repack).
  * exp is the true bottleneck (25.2M psum floats must cross ACT/DVE at
    1 elem/lane/cycle): split per key-tile between ACT (table exp ->
    bf16) and DVE (Schraudolph: y=int16(l*128/ln2 + 16256) bitcast bf16,
    one tensor_scalar op). GPSIMD/DMA cannot read PSUM, so 2-way only.
  * AV uses the exp tile as the matmul *stationary* operand with the
    tiny V' [128 keys, 5] moving: out [128 queries, 4 o + 1 den], psum-
    accumulated over the 32 key tiles. Normalization is then per-
    partition: reciprocal [128,1] + tensor_scalar mult by scalar AP.
  * o [q,d] -> o^T via one XBAR transpose DMA per block ([128,128]) +
    24 partition-shift DMAs, then 9 shifted tap-copies (DMA) build a
    36-partition stacked o^T so the output conv is ONE matmul per chunk:
    out[64, 400] = wo36[36, 64].T @ oo36[36, window].
"""

import numpy as np

H, W, CIN, DM, NH, DH = 128, 48, 64, 32, 8, 4
HP, WP = H + 2, W + 2          # padded spatial dims for 3x3 SAME conv
PADN = HP * WP + 4             # padded flat buffer size (+4 tail slack)
NPIX = H * W                   # 6144
QW, KW = 24, 32                # per-block query/key column widths
NQ = H * QW                    # 3072 queries per block
NK = H * KW                    # 4096 keys per block
NKT = 32                       # key tiles (128 keys each) per block
G = 1024                       # query granule (logits psum tile width)
NG = NQ // G                   # 3
QC = 128                       # AV query chunk (psum partitions)
NQC = G // QC                  # 8 chunks per granule
CHUNK_ROWS = 8                 # conv output rows per matmul chunk
NCHUNK = H // CHUNK_ROWS       # 16
CN = CHUNK_ROWS * WP           # conv matmul free size, 400
OOF = 4                        # oo36 head slack (tap shifts can hit -1)

SCH_A = 128.0 / float(np.log(2.0))   # Schraudolph scale (bf16 pattern)
SCH_B = 127.0 * 128.0                # Schraudolph exponent-bias offset

# exp engine split: ACT is faster per element but also stages the output
# conv; DVE carries norm + conv-psum drains. ~4:3 keeps them balanced.
def _exp_engine(i):
    return 'A' if (i * 4) % 7 < 4 else 'D'

_cached = {}


def _build_nc():
    import concourse.bacc as bacc
    import concourse.tile as tile
    import concourse.mybir as mybir

    f32 = mybir.dt.float32
    bf16 = mybir.dt.bfloat16
    i16 = mybir.dt.int16

    nc = bacc.Bacc("TRN2", target_bir_lowering=False)

    xx_d = nc.dram_tensor("xx", [128, PADN], bf16, kind="ExternalInput")
    wqkv_d = nc.dram_tensor("wqkv", [128, 6 * 12], bf16, kind="ExternalInput")
    bias_d = nc.dram_tensor("bias12", [12, 1], f32, kind="ExternalInput")
    wo36_d = nc.dram_tensor("wo36", [36, 64], bf16, kind="ExternalInput")
    id4_d = nc.dram_tensor("id4", [DH, DH], bf16, kind="ExternalInput")
    outp_d = nc.dram_tensor("outp", [CIN, NPIX], f32, kind="ExternalOutput")

    with tile.TileContext(nc) as tc:
        with tc.tile_pool(name="main", bufs=1) as mp:
            wqkv = mp.tile([128, 6 * 12], bf16)
            bias12 = mp.tile([12, 1], f32)
            wo36 = mp.tile([36, 64], bf16)
            id4 = mp.tile([DH, DH], bf16)
            kTb = mp.tile([DH, 2 * NK], bf16)    # block-contiguous key strips
            qb = mp.tile([DH, 2 * NQ], bf16)     # block-contiguous queries
            vp = mp.tile([128, 2 * NKT * 5], bf16)  # V' tiles: 4 v cols + 1.0
            obT = mp.tile([DH, 2 * NQ], bf16)    # normalized o^T, block-major
            oo36 = mp.tile([36, OOF + PADN], bf16)  # 9-tap stacked padded o^T
            actwarm = mp.tile([128, 1], f32)     # exp table preload target
            warmsrc = mp.tile([128, 1], f32)
            pewarm = mp.tile([DH, 512], bf16)    # garbage src for PE warmup
            # first exp-weight buffer lives in the persistent pool so
            # granule-0 exp can interleave with the conv
            exA = mp.tile([128, NKT * G], bf16)
            nc.vector.memset(warmsrc[:], -30.0)
            # dummy exp at t~0 pulls the ACT exp-table load off the
            # critical path
            nc.scalar.activation(
                actwarm[:], warmsrc[:], mybir.ActivationFunctionType.Exp,
            )
            nc.vector.memset(vp[:], 1.0)
            nc.gpsimd.memset(oo36[:], 0.0)

            nc.sync.dma_start(wqkv[:], wqkv_d.ap())
            nc.sync.dma_start(bias12[:], bias_d.ap())
            nc.sync.dma_start(wo36[:], wo36_d.ap())
            nc.sync.dma_start(id4[:], id4_d.ap())

            qb_v = qb[:].rearrange("p (b h w) -> p b h w", b=2, w=QW)

            # ---- q/k/v projections ----
            with tc.tile_pool(name="cvp", bufs=1) as cvp:
                xx = cvp.tile([128, PADN], bf16)
                qkvT = cvp.tile([12, NPIX], bf16)
                vTb = cvp.tile([DH, 2 * NK], bf16)
                xx_ap = xx_d.ap()
                for q4 in range(4):
                    s4 = (PADN // 4) * q4
                    e4 = PADN if q4 == 3 else (PADN // 4) * (q4 + 1)
                    nc.sync.dma_start(xx[:, s4:e4], xx_ap[:, s4:e4])

                # ~3.4us of dummy matmuls during the DMA-in window warms
                # the PE p-state before the conv (reads uninit SBUF)
                with tc.tile_pool(name="wps", bufs=1, space="PSUM") as wps:
                    nc.vector.memset(pewarm[:], 1.0)
                    wp = wps.tile([DH, 512], f32, tag="wp")
                    for _ in range(6):
                        nc.tensor.matmul(wp[:], pewarm[:, 0:DH], pewarm[:],
                                         start=True, stop=True)

                qT_v = qkvT[0:4, :].rearrange("p (h w) -> p h w", w=W)

                def lg_exp(b, g, kt, ex, pool):
                    lg = pool.tile([128, G], f32, tag="lg", name="lg")
                    kap = kTb[:, b * NK + 128 * kt:b * NK + 128 * (kt + 1)]
                    q0 = b * NQ + g * G
                    for j in range(G // 512):
                        nc.tensor.matmul(
                            lg[:, 512 * j:512 * (j + 1)],
                            kap,
                            qb[:, q0 + 512 * j:q0 + 512 * (j + 1)],
                            start=True, stop=True,
                        )
                    eng = _exp_engine((b * NG + g) * NKT + kt)
                    dst = ex[:, kt * G:(kt + 1) * G]
                    if eng == 'A':
                        nc.scalar.activation(
                            dst, lg[:], mybir.ActivationFunctionType.Exp)
                    else:
                        nc.vector.tensor_scalar(
                            out=dst.bitcast(i16), in0=lg[:],
                            scalar1=SCH_A, scalar2=SCH_B,
                            op0=mybir.AluOpType.mult,
                            op1=mybir.AluOpType.add,
                        )

                with (
                    tc.tile_pool(name="cps", bufs=1, space="PSUM") as cps,
                    tc.tile_pool(name="tps", bufs=1, space="PSUM") as tps,
                    tc.tile_pool(name="lg0p", bufs=2, space="PSUM") as lg0p,
                ):
                    for ci in range(NCHUNK):
                        ps = cps.tile([12, CN], f32, tag="cps")
                        f0 = ci * CHUNK_ROWS * WP
                        for dh in range(3):
                            s = f0 + dh * WP
                            nc.tensor.matmul(
                                ps[:], wqkv[:, 12 * dh:12 * (dh + 1)],
                                xx[:, s:s + CN],
                                start=(dh == 0), stop=False,
                            )
                            nc.tensor.matmul(
                                ps[:],
                                wqkv[0:CIN, 36 + 12 * dh:36 + 12 * (dh + 1)],
                                xx[0:CIN, s + 2:s + 2 + CN],
                                start=False, stop=(dh == 2),
                            )
                        # bias add + junk-column drop (cast to bf16)
                        psv = ps[:].rearrange("p (r c) -> p r c", c=WP)
                        nc.vector.tensor_scalar_add(
                            qkvT[:, ci * CHUNK_ROWS * W:(ci + 1) * CHUNK_ROWS * W],
                            psv[:, :, 0:W], bias12[:])
                        r0 = ci * CHUNK_ROWS
                        rr = slice(r0, r0 + CHUNK_ROWS)
                        # queries into block-contiguous layout (Pool engine)
                        for b in range(2):
                            nc.gpsimd.tensor_copy(
                                qb_v[:, b, rr], qT_v[:, rr, QW * b:QW * b + QW])
                        if ci % 4 == 3:
                            # k/v block-strip extraction (partition shift +
                            # column reshuffle) + V' transposes for the 8
                            # key tiles covered by this 4-chunk group
                            gi4 = ci // 4
                            qk_v = qkvT[:].rearrange("p (h w) -> p h w", w=W)
                            rsl = slice(32 * gi4, 32 * gi4 + 32)
                            for b in range(2):
                                dsl = slice(b * NK + 1024 * gi4,
                                            b * NK + 1024 * (gi4 + 1))
                                nc.sync.dma_start(
                                    kTb[:, dsl],
                                    qk_v[4:8, rsl, 16 * b:16 * b + KW])
                                nc.sync.dma_start(
                                    vTb[:, dsl],
                                    qk_v[8:12, rsl, 16 * b:16 * b + KW])
                            kt0 = gi4 * 8
                            for b in range(2):
                                tg = tps.tile([128, 32], bf16, tag="tg")
                                for j in range(8):
                                    kt = kt0 + j
                                    nc.tensor.transpose(
                                        tg[:, 4 * j:4 * j + 4],
                                        vTb[:, b * NK + 128 * kt:
                                            b * NK + 128 * (kt + 1)],
                                        id4[:],
                                    )
                                dst = vp[:].rearrange("p (t c) -> p t c", c=5)
                                src = tg[:].rearrange("p (t c) -> p t c", c=4)
                                nc.vector.tensor_copy(
                                    dst[:, b * NKT + kt0:b * NKT + kt0 + 8, 0:4],
                                    src[:])
                            # granule-0 (b=0) logits+exp whose k rows and
                            # queries are already produced
                            gi = ci // 4
                            for kt in {1: range(0, 16), 2: range(16, 24),
                                       3: range(24, 32)}.get(gi, ()):
                                lg_exp(0, 0, kt, exA, lg0p)

            # ---- attention ----
            # per granule (b, g): 32 key tiles: logits -> exp (ACT or DVE)
            # into exbuf; AV of the PREVIOUS granule interleaves so its
            # matmuls fill PE stalls while exp catches up.
            with (
                tc.tile_pool(name="exp", bufs=1) as exp_pool,
                tc.tile_pool(name="lgp", bufs=3, space="PSUM") as lgp,
                tc.tile_pool(name="o5p", bufs=2, space="PSUM") as o5p,
                tc.tile_pool(name="dnp", bufs=2) as dnp,
                tc.tile_pool(name="onp", bufs=2) as onp,
                tc.tile_pool(name="stk", bufs=2) as stk,
            ):
                # second exp-weight buffer (alternates with exA so
                # exp(g+1) overlaps AV(g))
                exB = exp_pool.tile([128, NKT * G], bf16)
                grans = [(b, g) for b in range(2) for g in range(NG)]
                o4nb = {}

                def av_granule(bg, qc):
                    b, g = bg
                    ex = exA if (b * NG + g) % 2 == 0 else exB
                    o5 = o5p.tile([QC, 5], f32, tag="o5")
                    for kt in range(NKT):
                        nc.tensor.matmul(
                            o5[:],
                            ex[:, kt * G + qc * QC: kt * G + qc * QC + QC],
                            vp[:, (b * NKT + kt) * 5:(b * NKT + kt) * 5 + 5],
                            start=(kt == 0), stop=(kt == NKT - 1),
                        )
                    dn = dnp.tile([QC, 1], f32, tag="dn")
                    nc.vector.reciprocal(dn[:], o5[:, 4:5])
                    c0 = 4 * (g * NQC + qc)
                    nc.vector.tensor_scalar(
                        out=o4nb[b][:, c0:c0 + 4], in0=o5[:, 0:4],
                        scalar1=dn[:], scalar2=None,
                        op0=mybir.AluOpType.mult,
                    )

                def flush_block(b):
                    # o4nb [128 q, 96(+pad)] -> stacked [rows 4c+d] -> obT
                    st = stk.tile([128, 128], bf16, tag="st")
                    nc.sync.dma_start_transpose(st[:], o4nb[b][:])
                    nqc_b = NQ // QC
                    for d in range(4):
                        dst = obT[d:d + 1, b * NQ:(b + 1) * NQ].rearrange(
                            "p (c q) -> p c q", q=QC)
                        nc.sync.dma_start(dst, st[d:4 * nqc_b:4, 0:QC])
                    # 9 shifted tap copies into the 36-partition stack
                    obv = obT[:].rearrange("p (b r w) -> p b r w", b=2, w=QW)
                    for dh in range(3):
                        for dw in range(3):
                            t = 3 * dh + dw
                            r0 = 1 if dh == 2 else 0
                            nr = H - r0
                            off = (OOF + (1 + r0 - dh) * WP
                                   + 24 * b + 1 - dw)
                            dst = oo36[4 * t:4 * t + 4, off:off + nr * WP]
                            dst = dst.rearrange(
                                "p (r w) -> p r w", w=WP)[:, :, 0:QW]
                            nc.sync.dma_start(dst, obv[:, b, r0:r0 + nr])

                o4nb[0] = onp.tile([128, 128], bf16, name="o4nb", tag="o4nb")
                prev = (0, 0)
                for bg in grans[1:]:
                    b, g = bg
                    if g == 0:
                        o4nb[b] = onp.tile([128, 128], bf16, name="o4nb", tag="o4nb")
                    ex = exA if (b * NG + g) % 2 == 0 else exB
                    for kt in range(NKT):
                        lg = lgp.tile([128, G], f32, tag="lg", name="lg")
                        kap = kTb[:, b * NK + 128 * kt:b * NK + 128 * (kt + 1)]
                        q0 = b * NQ + g * G
                        for j in range(G // 512):
                            nc.tensor.matmul(
                                lg[:, 512 * j:512 * (j + 1)],
                                kap,
                                qb[:, q0 + 512 * j:q0 + 512 * (j + 1)],
                                start=True, stop=True,
                            )
                        eng = _exp_engine((b * NG + g) * NKT + kt)
                        dst = ex[:, kt * G:(kt + 1) * G]
                        if eng == 'A':
                            nc.scalar.activation(
                                dst, lg[:], mybir.ActivationFunctionType.Exp)
                        else:
                            nc.vector.tensor_scalar(
                                out=dst.bitcast(i16), in0=lg[:],
                                scalar1=SCH_A, scalar2=SCH_B,
                                op0=mybir.AluOpType.mult,
                                op1=mybir.AluOpType.add,
                            )
                        if kt % 4 == 3:
                            av_granule(prev, kt // 4)
                    if prev[1] == NG - 1:
                        flush_block(prev[0])
                    prev = bg
                for qc in range(NQC):
                    av_granule(prev, qc)
                flush_block(prev[0])

            # ---- output conv: one matmul per chunk over the 36-row stack
            outp_ap = outp_d.ap()
            with (
                tc.tile_pool(name="ops", bufs=3, space="PSUM") as ops,
                tc.tile_pool(name="ost", bufs=3) as ost,
            ):
                for c2 in range(NCHUNK // 2):
                    stage = ost.tile([CIN, 2 * CHUNK_ROWS * W], f32, tag="ost")
                    for half in range(2):
                        ci = 2 * c2 + half
                        ps = ops.tile([CIN, CN], f32, tag="ops")
                        f0 = OOF + ci * CHUNK_ROWS * WP
                        nc.tensor.matmul(
                            ps[:], wo36[:], oo36[:, f0:f0 + CN],
                            start=True, stop=True,
                        )
                        psv = ps[:].rearrange("p (r c) -> p r c", c=WP)
                        dst = stage[:, half * CHUNK_ROWS * W:
                                    (half + 1) * CHUNK_ROWS * W]
                        if half == 0:
                            nc.scalar.copy(dst, psv[:, :, 0:W])
                        else:
                            nc.vector.tensor_copy(dst, psv[:, :, 0:W])
                    nc.sync.dma_start(
                        outp_ap[:, 2 * c2 * CHUNK_ROWS * W:
                                (2 * c2 + 2) * CHUNK_ROWS * W],
                        stage[:])

    nc.compile()
    return nc


def _prep_inputs(x, wq, bq, wk, bk, wv, bv, wo):
    f32 = np.float32
    x = np.ascontiguousarray(np.asarray(x, f32))
    scale = f32(DH) ** -0.5

    bf = ml_bf16()
    xx = np.zeros((128, PADN), np.float32)
    xv = xx[:CIN, :HP * WP].reshape(CIN, HP, WP)
    xv[:, 1:1 + H, 1:1 + W] = x[0].transpose(2, 0, 1)
    xx[CIN:, :PADN - 1] = xx[:CIN, 1:]
    xx = xx.astype(bf)

    wq = np.asarray(wq, f32) * scale
    bq = np.asarray(bq, f32) * scale
    wk = np.asarray(wk, f32)
    bk = np.asarray(bk, f32)
    wv = np.asarray(wv, f32)
    bv = np.asarray(bv, f32)
    wo = np.asarray(wo, f32)

    id4 = np.eye(DH, dtype=bf)
    in_maps = []
    for h in range(NH):
        sl = slice(4 * h, 4 * h + 4)
        wqkv = np.zeros((128, 6, 12), f32)
        for dh in range(3):
            for p, dw in ((0, 0), (1, 1)):   # pair slots on partition halves
                wqkv[64 * p:64 * p + CIN, dh, 0:4] = wq[dh, dw, :, sl]
                wqkv[64 * p:64 * p + CIN, dh, 4:8] = wk[dh, dw, :, sl]
                wqkv[64 * p:64 * p + CIN, dh, 8:12] = wv[dh, dw, :, sl]
            wqkv[:CIN, 3 + dh, 0:4] = wq[dh, 2, :, sl]
            wqkv[:CIN, 3 + dh, 4:8] = wk[dh, 2, :, sl]
            wqkv[:CIN, 3 + dh, 8:12] = wv[dh, 2, :, sl]
        bias12 = np.concatenate([bq[sl], bk[sl], bv[sl]]).reshape(12, 1)
        wo36 = np.zeros((36, 64), f32)
        for dh in range(3):
            for dw in range(3):
                t = 3 * dh + dw
                wo36[4 * t:4 * t + 4, :] = wo[dh, dw, sl, :]
        in_maps.append({
            "xx": xx,
            "bias12": np.ascontiguousarray(bias12.astype(f32)),
            "wqkv": np.ascontiguousarray(wqkv.reshape(128, 6 * 12).astype(bf)),
            "wo36": np.ascontiguousarray(wo36.astype(bf)),
            "id4": id4,
        })
    return in_maps


def ml_bf16():
    import ml_dtypes
    return ml_dtypes.bfloat16


def _run(in_maps, trace=False, trace_cores=None):
    from concourse.bass_utils import run_bass_kernel_spmd

    if "nc" not in _cached:
        _cached["nc"] = _build_nc()
    return run_bass_kernel_spmd(
        _cached["nc"], in_maps, core_ids=list(range(NH)),
        trace=trace, trace_cores=trace_cores,
    )


def kernel(x, wq, bq, wk, bk, wv, bv, wo):
    in_maps = _prep_inputs(x, wq, bq, wk, bk, wv, bv, wo)
    res = _run(in_maps)
    acc = np.zeros((CIN, NPIX), np.float64)
    for r in res.results:
        acc += r["outp"].astype(np.float64)
    out = acc.astype(np.float32).reshape(CIN, H, W).transpose(1, 2, 0)
    return out[None]


# revision 37
# speedup vs baseline: 1.4959x; 1.1085x over previous
"""Trainium2 Bass kernel for 2D block-local multi-head attention.

Problem (hardcoded): x [1,128,48,64] -> 3x3 conv projections to q/k/v
(d_model=32, 8 heads, d_head=4), t2t local_attention_2d with
query_shape=(128,24), memory_flange=(8,8), combine heads, 3x3 output conv.

Structural facts exploited:
  * H=128, W=48, query blocks 128x24 -> exactly 2 blocks (nH=1, nW=2).
  * The memory flange is entirely zero padding, masked to -1e9 by the
    reference (exp -> exactly 0), so block b's effective key set is the
    static 128x32 strip of ORIGINAL pixels: cols [16b, 16b+32).
  * bf16 exp weights keep softmax exact enough (fp8 weights and fp8 q/k
    both blow the 2e-2 budget; measured empirically).

Sharding: one head per NeuronCore, zero cross-core communication. Each
core computes q/k/v (full image), block-local attention, and a partial
output conv over its 4 head channels; host sums the 8 partial results.

Performance structure (per core):
  * Conv qkv: channel-major on the padded 130x50 grid; a 3x3 tap is a
    flat-index shift. 5 matmul passes per chunk: 3 tap-pairs on xx
    (rows 64:128 = col+1 copy) + the (0,2)/(1,2) pair on xx2 (pre-
    shifted +2 / +WP+2) + the (2,2) single. Granule-0 logits+exp
    interleave into the conv window (own 3-buffer psum pool).
  * Logits: out = K_tile.T @ Q -> psum [128 keys, 1024 queries]; K tile
    is a strided view straight into the spatial k^T image (no repack).
  * exp is the true bottleneck (25.2M psum floats must cross ACT/DVE at
    1 elem/lane/cycle): split per key-tile between ACT (table exp ->
    bf16) and DVE (Schraudolph: y=int16(l*128/ln2 + 16256) bitcast bf16,
    one tensor_scalar op). GPSIMD/DMA cannot read PSUM, so 2-way only.
  * AV uses the exp tile as the matmul *stationary* operand with the
    tiny V' [128 keys, 5] moving: out [128 queries, 4 o + 1 den], psum-
    accumulated over the 32 key tiles. Normalization is then per-
    partition: reciprocal [128,1] + tensor_scalar mult by scalar AP.
  * o [q,d] -> o^T via one XBAR transpose DMA per block ([128,128]) +
    24 partition-shift DMAs, then 9 shifted tap-copies (DMA) build a
    36-partition stacked o^T so the output conv is ONE matmul per chunk:
    out[64, 400] = wo36[36, 64].T @ oo36[36, window].
"""

import numpy as np

H, W, CIN, DM, NH, DH = 128, 48, 64, 32, 8, 4
HP, WP = H + 2, W + 2          # padded spatial dims for 3x3 SAME conv
PADN = HP * WP + 4             # padded flat buffer size (+4 tail slack)
NPIX = H * W                   # 6144
QW, KW = 24, 32                # per-block query/key column widths
NQ = H * QW                    # 3072 queries per block
NK = H * KW                    # 4096 keys per block
NKT = 32                       # key tiles (128 keys each) per block
G = 1024                       # query granule (logits psum tile width)
NG = NQ // G                   # 3
QC = 128                       # AV query chunk (psum partitions)
NQC = G // QC                  # 8 chunks per granule
CHUNK_ROWS = 8                 # conv output rows per matmul chunk
NCHUNK = H // CHUNK_ROWS       # 16
CN = CHUNK_ROWS * WP           # conv matmul free size, 400
OOF = 4                        # oo36 head slack (tap shifts can hit -1)

SCH_A = 128.0 / float(np.log(2.0))   # Schraudolph scale (bf16 pattern)
SCH_B = 127.0 * 128.0                # Schraudolph exponent-bias offset

# exp engine split: ~0.56 of tiles on ACT (1038ns/tile incl overheads)
# vs DVE Schraudolph (1192ns/tile); DVE also carries norms + conv drains.
def _exp_engine(i):
    if i < 32:
        return 'D' if i % 4 != 3 else 'A'
    return 'A' if (i * 16) % 25 < 16 else 'D'

_cached = {}


def _build_nc():
    import concourse.bacc as bacc
    import concourse.tile as tile
    import concourse.mybir as mybir

    f32 = mybir.dt.float32
    bf16 = mybir.dt.bfloat16
    i16 = mybir.dt.int16

    nc = bacc.Bacc("TRN2", target_bir_lowering=False)

    xx_d = nc.dram_tensor("xx", [128, PADN], bf16, kind="ExternalInput")
    xx2_d = nc.dram_tensor("xx2", [128, PADN], bf16, kind="ExternalInput")
    wqkv_d = nc.dram_tensor("wqkv", [128, 5 * 12], bf16, kind="ExternalInput")
    bias_d = nc.dram_tensor("bias12", [12, 1], f32, kind="ExternalInput")
    wo36_d = nc.dram_tensor("wo36", [36, 64], bf16, kind="ExternalInput")
    id4_d = nc.dram_tensor("id4", [DH, DH], bf16, kind="ExternalInput")
    outp_d = nc.dram_tensor("outp", [CIN, NPIX], f32, kind="ExternalOutput")

    with tile.TileContext(nc) as tc:
        with tc.tile_pool(name="main", bufs=1) as mp:
            wqkv = mp.tile([128, 5 * 12], bf16)
            bias12 = mp.tile([12, 1], f32)
            wo36 = mp.tile([36, 64], bf16)
            id4 = mp.tile([DH, DH], bf16)
            kTb = mp.tile([DH, 2 * NK], bf16)    # block-contiguous key strips
            qb = mp.tile([DH, 2 * NQ], bf16)     # block-contiguous queries
            vp = mp.tile([128, 2 * NKT * 5], bf16)  # V' tiles: 4 v cols + 1.0
            obT = mp.tile([DH, 2 * NQ], bf16)    # normalized o^T, block-major
            oo36 = mp.tile([36, OOF + PADN], bf16)  # 9-tap stacked padded o^T
            actwarm = mp.tile([128, 1], f32)     # exp table preload target
            pewarm = mp.tile([DH, 512], bf16)    # garbage src for PE warmup
            warmsrc = mp.tile([128, 1], f32)
            # first exp-weight buffer lives in the persistent pool so
            # granule-0 exp can interleave with the conv
            exA = mp.tile([128, NKT * G], bf16)
            nc.vector.memset(warmsrc[:], -30.0)
            # dummy exp at t~0 pulls the ACT exp-table load off the
            # critical path
            nc.scalar.activation(
                actwarm[:], warmsrc[:], mybir.ActivationFunctionType.Exp,
            )
            nc.vector.memset(vp[:], 1.0)
            nc.gpsimd.memset(oo36[:], 0.0)

            nc.sync.dma_start(wqkv[:], wqkv_d.ap())
            nc.sync.dma_start(bias12[:], bias_d.ap())
            nc.sync.dma_start(wo36[:], wo36_d.ap())
            nc.sync.dma_start(id4[:], id4_d.ap())

            qb_v = qb[:].rearrange("p (b h w) -> p b h w", b=2, w=QW)

            # ---- q/k/v projections ----
            with tc.tile_pool(name="cvp", bufs=1) as cvp:
                xx = cvp.tile([128, PADN], bf16)
                xx2 = cvp.tile([128, PADN], bf16)
                qkvT = cvp.tile([12, NPIX], bf16)
                vTb = cvp.tile([DH, 2 * NK], bf16)
                xx_ap = xx_d.ap()
                xx2_ap = xx2_d.ap()
                for q4 in range(4):
                    s4 = (PADN // 4) * q4
                    e4 = PADN if q4 == 3 else (PADN // 4) * (q4 + 1)
                    nc.sync.dma_start(xx[:, s4:e4], xx_ap[:, s4:e4])
                    nc.sync.dma_start(xx2[:, s4:e4], xx2_ap[:, s4:e4])

                # dummy matmuls burn the cost model's 3us p-state ramp
                # during the DMA-in window (reads uninit SBUF, result unused)
                with tc.tile_pool(name="wps", bufs=1, space="PSUM") as wps:
                    nc.vector.memset(pewarm[:], 1.0)
                    wp = wps.tile([DH, 512], f32, tag="wp")
                    for _ in range(5):
                        nc.tensor.matmul(wp[:], pewarm[:, 0:DH], pewarm[:],
                                         start=True, stop=True)

                qT_v = qkvT[0:4, :].rearrange("p (h w) -> p h w", w=W)

                def lg_exp(b, g, kt, ex, pool):
                    lg = pool.tile([128, G], f32, tag="lg", name="lg")
                    kap = kTb[:, b * NK + 128 * kt:b * NK + 128 * (kt + 1)]
                    q0 = b * NQ + g * G
                    for j in range(G // 512):
                        nc.tensor.matmul(
                            lg[:, 512 * j:512 * (j + 1)],
                            kap,
                            qb[:, q0 + 512 * j:q0 + 512 * (j + 1)],
                            start=True, stop=True,
                        )
                    eng = _exp_engine((b * NG + g) * NKT + kt)
                    dst = ex[:, kt * G:(kt + 1) * G]
                    if eng == 'A':
                        nc.scalar.activation(
                            dst, lg[:], mybir.ActivationFunctionType.Exp)
                    else:
                        nc.vector.tensor_scalar(
                            out=dst.bitcast(i16), in0=lg[:],
                            scalar1=SCH_A, scalar2=SCH_B,
                            op0=mybir.AluOpType.mult,
                            op1=mybir.AluOpType.add,
                        )

                with (
                    tc.tile_pool(name="cps", bufs=2, space="PSUM") as cps,
                    tc.tile_pool(name="tps", bufs=1, space="PSUM") as tps,
                    tc.tile_pool(name="lg0p", bufs=2, space="PSUM") as lg0p,
                ):
                    for ci in range(NCHUNK):
                        ps = cps.tile([12, CN], f32, tag="cps")
                        f0 = ci * CHUNK_ROWS * WP
                        for dh in range(3):
                            nc.tensor.matmul(
                                ps[:], wqkv[:, 12 * dh:12 * (dh + 1)],
                                xx[:, f0 + dh * WP:f0 + dh * WP + CN],
                                start=(dh == 0), stop=False,
                            )
                        # taps (0,2)+(1,2) packed on xx2 (pre-shifted +2 and
                        # +WP+2 on the partition halves)
                        nc.tensor.matmul(
                            ps[:], wqkv[:, 36:48], xx2[:, f0:f0 + CN],
                            start=False, stop=False,
                        )
                        nc.tensor.matmul(
                            ps[:], wqkv[0:CIN, 48:60],
                            xx[0:CIN, f0 + 2 * WP + 2:f0 + 2 * WP + 2 + CN],
                            start=False, stop=True,
                        )
                        # bias add + junk-column drop (cast to bf16)
                        psv = ps[:].rearrange("p (r c) -> p r c", c=WP)
                        nc.vector.tensor_scalar_add(
                            qkvT[:, ci * CHUNK_ROWS * W:(ci + 1) * CHUNK_ROWS * W],
                            psv[:, :, 0:W], bias12[:])
                        r0 = ci * CHUNK_ROWS
                        rr = slice(r0, r0 + CHUNK_ROWS)
                        # queries into block-contiguous layout (Pool engine)
                        for b in range(2):
                            nc.gpsimd.tensor_copy(
                                qb_v[:, b, rr], qT_v[:, rr, QW * b:QW * b + QW])
                        if ci % 4 == 3:
                            # k/v block-strip extraction (partition shift +
                            # column reshuffle) + V' transposes for the 8
                            # key tiles covered by this 4-chunk group
                            gi4 = ci // 4
                            qk_v = qkvT[:].rearrange("p (h w) -> p h w", w=W)
                            rsl = slice(32 * gi4, 32 * gi4 + 32)
                            for b in range(2):
                                dsl = slice(b * NK + 1024 * gi4,
                                            b * NK + 1024 * (gi4 + 1))
                                nc.sync.dma_start(
                                    kTb[:, dsl],
                                    qk_v[4:8, rsl, 16 * b:16 * b + KW])
                                nc.sync.dma_start(
                                    vTb[:, dsl],
                                    qk_v[8:12, rsl, 16 * b:16 * b + KW])
                            # granule-0 (b=0) logits+exp whose k rows and
                            # queries are already produced
                            gi = ci // 4
                            for kt in {1: range(0, 8), 2: range(8, 16)}.get(
                                    gi, ()):
                                lg_exp(0, 0, kt, exA, lg0p)

                # V' build: transposes of all 64 key tiles (vTb still live)
                with tc.tile_pool(name="tps", bufs=2, space="PSUM") as tps:
                    for b in range(2):
                        for kt8 in range(0, NKT, 8):
                            tg = tps.tile([128, 32], bf16, tag="tg")
                            for j in range(8):
                                kt = kt8 + j
                                nc.tensor.transpose(
                                    tg[:, 4 * j:4 * j + 4],
                                    vTb[:, b * NK + 128 * kt:
                                        b * NK + 128 * (kt + 1)],
                                    id4[:],
                                )
                            dstv = vp[:].rearrange("p (t c) -> p t c", c=5)
                            srcv = tg[:].rearrange("p (t c) -> p t c", c=4)
                            nc.vector.tensor_copy(
                                dstv[:, b * NKT + kt8:b * NKT + kt8 + 8, 0:4],
                                srcv[:])

            # ---- attention ----
            # per granule (b, g): 32 key tiles: logits -> exp (ACT or DVE)
            # into exbuf; AV of the PREVIOUS granule interleaves so its
            # matmuls fill PE stalls while exp catches up.
            with (
                tc.tile_pool(name="exp", bufs=1) as exp_pool,
                tc.tile_pool(name="lgp", bufs=3, space="PSUM") as lgp,
                tc.tile_pool(name="o5p", bufs=2, space="PSUM") as o5p,
                tc.tile_pool(name="dnp", bufs=2) as dnp,
                tc.tile_pool(name="onp", bufs=2) as onp,
                tc.tile_pool(name="stk", bufs=2) as stk,
            ):
                # second exp-weight buffer (alternates with exA so
                # exp(g+1) overlaps AV(g))
                exB = exp_pool.tile([128, NKT * G], bf16)
                grans = [(b, g) for b in range(2) for g in range(NG)]
                o4nb = {}

                def av_granule(bg, qc):
                    b, g = bg
                    ex = exA if (b * NG + g) % 2 == 0 else exB
                    o5 = o5p.tile([QC, 5], f32, tag="o5")
                    for kt in range(NKT):
                        nc.tensor.matmul(
                            o5[:],
                            ex[:, kt * G + qc * QC: kt * G + qc * QC + QC],
                            vp[:, (b * NKT + kt) * 5:(b * NKT + kt) * 5 + 5],
                            start=(kt == 0), stop=(kt == NKT - 1),
                        )
                    dn = dnp.tile([QC, 1], f32, tag="dn")
                    nc.vector.reciprocal(dn[:], o5[:, 4:5])
                    c0 = 4 * (g * NQC + qc)
                    nc.vector.tensor_scalar(
                        out=o4nb[b][:, c0:c0 + 4], in0=o5[:, 0:4],
                        scalar1=dn[:], scalar2=None,
                        op0=mybir.AluOpType.mult,
                    )

                def flush_block(b, use_act=False):
                    # o4nb [128 q, 96(+pad)] -> stacked [rows 4c+d] -> obT
                    st = stk.tile([128, 128], bf16, tag="st")
                    nc.sync.dma_start_transpose(st[:], o4nb[b][:])
                    nqc_b = NQ // QC
                    for d in range(4):
                        dst = obT[d:d + 1, b * NQ:(b + 1) * NQ].rearrange(
                            "p (c q) -> p c q", q=QC)
                        nc.sync.dma_start(dst, st[d:4 * nqc_b:4, 0:QC])
                    # 9 shifted tap copies into the 36-partition stack; the
                    # tail flush is row-staged so the out conv can start on
                    # the top half while the bottom taps stream
                    obv = obT[:].rearrange("p (b r w) -> p b r w", b=2, w=QW)
                    halves = ((0, H),)
                    for rlo, rhi in halves:
                        for dh in range(3):
                            for dw in range(3):
                                t = 3 * dh + dw
                                qeng = (nc.scalar if use_act and
                                        t % 2 == 1 else nc.sync)
                                r0 = max(rlo, 1 if dh == 2 else 0)
                                nr = rhi - r0
                                off = (OOF + (1 + r0 - dh) * WP
                                       + 24 * b + 1 - dw)
                                dst = oo36[4 * t:4 * t + 4, off:off + nr * WP]
                                dst = dst.rearrange(
                                    "p (r w) -> p r w", w=WP)[:, :, 0:QW]
                                qeng.dma_start(dst, obv[:, b, r0:r0 + nr])

                o4nb[0] = onp.tile([128, 128], bf16, name="o4nb", tag="o4nb")
                for kt in range(16, NKT):
                    lg_exp(0, 0, kt, exA, lgp)
                prev = (0, 0)
                for bg in grans[1:]:
                    b, g = bg
                    if g == 0:
                        o4nb[b] = onp.tile([128, 128], bf16, name="o4nb", tag="o4nb")
                    ex = exA if (b * NG + g) % 2 == 0 else exB
                    for kt in range(NKT):
                        lg = lgp.tile([128, G], f32, tag="lg", name="lg")
                        kap = kTb[:, b * NK + 128 * kt:b * NK + 128 * (kt + 1)]
                        q0 = b * NQ + g * G
                        for j in range(G // 512):
                            nc.tensor.matmul(
                                lg[:, 512 * j:512 * (j + 1)],
                                kap,
                                qb[:, q0 + 512 * j:q0 + 512 * (j + 1)],
                                start=True, stop=True,
                            )
                        eng = _exp_engine((b * NG + g) * NKT + kt)
                        dst = ex[:, kt * G:(kt + 1) * G]
                        if eng == 'A':
                            nc.scalar.activation(
                                dst, lg[:], mybir.ActivationFunctionType.Exp)
                        else:
                            nc.vector.tensor_scalar(
                                out=dst.bitcast(i16), in0=lg[:],
                                scalar1=SCH_A, scalar2=SCH_B,
                                op0=mybir.AluOpType.mult,
                                op1=mybir.AluOpType.add,
                            )
                        if kt % 4 == 3:
                            av_granule(prev, kt // 4)
                    if prev[1] == NG - 1:
                        flush_block(prev[0])
                    prev = bg
                for qc in range(NQC):
                    av_granule(prev, qc)
                flush_block(prev[0], use_act=True)

            # ---- output conv: one matmul per chunk over the 36-row stack
            outp_ap = outp_d.ap()
            with (
                tc.tile_pool(name="ops", bufs=4, space="PSUM") as ops,
                tc.tile_pool(name="ost", bufs=3) as ost,
            ):
                for c2 in range(NCHUNK // 2):
                    stage = ost.tile([CIN, 2 * CHUNK_ROWS * W], f32, tag="ost")
                    for half in range(2):
                        ci = 2 * c2 + half
                        ps = ops.tile([CIN, CN], f32, tag="ops")
                        f0 = OOF + ci * CHUNK_ROWS * WP
                        nc.tensor.matmul(
                            ps[:], wo36[:], oo36[:, f0:f0 + CN],
                            start=True, stop=True,
                        )
                        psv = ps[:].rearrange("p (r c) -> p r c", c=WP)
                        dst = stage[:, half * CHUNK_ROWS * W:
                                    (half + 1) * CHUNK_ROWS * W]
                        if half == 0:
                            nc.scalar.copy(dst, psv[:, :, 0:W])
                        else:
                            nc.vector.tensor_copy(dst, psv[:, :, 0:W])
                    nc.sync.dma_start(
                        outp_ap[:, 2 * c2 * CHUNK_ROWS * W:
                                (2 * c2 + 2) * CHUNK_ROWS * W],
                        stage[:])

    nc.compile()
    return nc


def _prep_inputs(x, wq, bq, wk, bk, wv, bv, wo):
    f32 = np.float32
    x = np.ascontiguousarray(np.asarray(x, f32))
    scale = f32(DH) ** -0.5

    bf = ml_bf16()
    xx = np.zeros((128, PADN), np.float32)
    xv = xx[:CIN, :HP * WP].reshape(CIN, HP, WP)
    xv[:, 1:1 + H, 1:1 + W] = x[0].transpose(2, 0, 1)
    xx[CIN:, :PADN - 1] = xx[:CIN, 1:]
    xx2 = np.zeros((128, PADN), np.float32)
    xx2[:CIN, :PADN - 2] = xx[:CIN, 2:]
    xx2[CIN:, :PADN - (WP + 2)] = xx[:CIN, WP + 2:]
    xx2 = xx2.astype(bf)
    xx = xx.astype(bf)

    wq = np.asarray(wq, f32) * scale
    bq = np.asarray(bq, f32) * scale
    wk = np.asarray(wk, f32)
    bk = np.asarray(bk, f32)
    wv = np.asarray(wv, f32)
    bv = np.asarray(bv, f32)
    wo = np.asarray(wo, f32)

    id4 = np.eye(DH, dtype=bf)
    in_maps = []
    for h in range(NH):
        sl = slice(4 * h, 4 * h + 4)
        wqkv = np.zeros((128, 5, 12), f32)
        for dh in range(3):
            for p, dw in ((0, 0), (1, 1)):   # pair slots on partition halves
                wqkv[64 * p:64 * p + CIN, dh, 0:4] = wq[dh, dw, :, sl]
                wqkv[64 * p:64 * p + CIN, dh, 4:8] = wk[dh, dw, :, sl]
                wqkv[64 * p:64 * p + CIN, dh, 8:12] = wv[dh, dw, :, sl]
        for p, dh in ((0, 0), (1, 1)):       # (0,2)+(1,2) pair on xx2 halves
            wqkv[64 * p:64 * p + CIN, 3, 0:4] = wq[dh, 2, :, sl]
            wqkv[64 * p:64 * p + CIN, 3, 4:8] = wk[dh, 2, :, sl]
            wqkv[64 * p:64 * p + CIN, 3, 8:12] = wv[dh, 2, :, sl]
        wqkv[:CIN, 4, 0:4] = wq[2, 2, :, sl]
        wqkv[:CIN, 4, 4:8] = wk[2, 2, :, sl]
        wqkv[:CIN, 4, 8:12] = wv[2, 2, :, sl]
        bias12 = np.concatenate([bq[sl], bk[sl], bv[sl]]).reshape(12, 1)
        wo36 = np.zeros((36, 64), f32)
        for dh in range(3):
            for dw in range(3):
                t = 3 * dh + dw
                wo36[4 * t:4 * t + 4, :] = wo[dh, dw, sl, :]
        in_maps.append({
            "xx": xx,
            "bias12": np.ascontiguousarray(bias12.astype(f32)),
            "wqkv": np.ascontiguousarray(wqkv.reshape(128, 5 * 12).astype(bf)),
            "xx2": xx2,
            "wo36": np.ascontiguousarray(wo36.astype(bf)),
            "id4": id4,
        })
    return in_maps


def ml_bf16():
    import ml_dtypes
    return ml_dtypes.bfloat16


def _run(in_maps, trace=False, trace_cores=None):
    from concourse.bass_utils import run_bass_kernel_spmd

    if "nc" not in _cached:
        _cached["nc"] = _build_nc()
    return run_bass_kernel_spmd(
        _cached["nc"], in_maps, core_ids=list(range(NH)),
        trace=trace, trace_cores=trace_cores,
    )


def kernel(x, wq, bq, wk, bk, wv, bv, wo):
    in_maps = _prep_inputs(x, wq, bq, wk, bk, wv, bv, wo)
    res = _run(in_maps)
    acc = np.zeros((CIN, NPIX), np.float64)
    for r in res.results:
        acc += r["outp"].astype(np.float64)
    out = acc.astype(np.float32).reshape(CIN, H, W).transpose(1, 2, 0)
    return out[None]
